# revision 11
# baseline (speedup 1.0000x reference)
"""Trainium2 Bass kernel: Conv2d(1->64, k=7, valid) on data [32,1,224,224] f32.

Data-parallel over batch (4 images per core on 8 cores).  Per core:
im2col matmul in fp16 (K=49 taps, M=64 out-channels), PSUM fp32, fp32 out.

Layout/pipeline (per core, 32 row-block "tiles" of 28 output rows):
  - host: fp16 cast; for each tile, SEVEN copies of its 34-row source
    block, copy ky pre-shifted down by ky rows.  Copies of one tile sit at
    partitions base+4*ky (7 distinct AXI ports); even tiles use the lower
    partition half / even ports, odd tiles the upper half / odd ports.
  - im2col: ONE SWDGE DMA per tile (3-dim AP): src dim0 walks the 7 slab
    copies, dim1 the 7 kx shifts (overlapping reads), dim2 a contiguous
    28*224-col run.  dst = [49, 6272] fp16 at partition base 0 (even
    tiles) or 64 (odd tiles).
  - matmul: pairs (even tile, odd tile): lhsT = W^T [49,64] fp16 at row
    base 0/64, out -> psum[0:64]/[64:128] of one bank.  Alternating row
    groups lets LDWEIGHTS overlap in-flight matmuls.
  - copy: psum [128,448] -> ob tile full width, DVE/ACT alternating.
  - out: one DMA per tile [64ch, 28*224 f32]; even tiles on the sync
    HWDGE ring, odd on scalar.  Cols 218..223 are garbage (kx wrap) and
    are sliced off on the host, as are rows >= 218.
"""

import numpy as np

B = 32            # full batch
OC = 64           # out channels
KS = 7            # kernel size
H = 224           # input H=W
OH = 218          # valid output rows/cols
OW = 224          # computed output width (incl 6 garbage cols)
NCORES = 8
IPC = B // NCORES  # images per core

BLK = 28          # output rows per tile
NBLK = 8          # tiles per image
SRC_ROWS = 34     # rows stored per slab copy
SLAB = SRC_ROWS * H + 8   # 7624 fp16 elements per slab
NTILES = IPC * NBLK       # 32 tiles per core
NPAIRS = NTILES // 2
NCOLS = BLK * OW          # 6272 im2col columns per tile
NMM = NCOLS // 448        # 14 matmuls per tile

# slab-group bases: tile t -> 7 slabs at partitions base+4*ky, where
# base = (64 if t odd) + BASES[(t//2) % 8], free slot (t//2) // 8.
BASES = [0, 1, 2, 3, 28, 29, 30, 31]

_CACHE = {}


def _tile_src(t):
    q = t // 2
    base = BASES[q % 8] + (64 if (t % 2) else 0)
    return base, q // 8  # partition base, slot


def _build():
    import concourse.bass as bass
    import concourse.mybir as mybir
    import concourse.tile as tile
    from concourse import bacc

    nc = bacc.Bacc("TRN2", target_bir_lowering=False, debug=False)

    xb = nc.dram_tensor("xb", [2, 128, SLAB], mybir.dt.float16,
                        kind="ExternalInput")
    wT = nc.dram_tensor("wT", [KS * KS, OC], mybir.dt.float16,
                        kind="ExternalInput")
    out = nc.dram_tensor("out", [IPC, OC, OH, OW], mybir.dt.float32,
                         kind="ExternalOutput")

    with tile.TileContext(nc) as tc:
        with (
            tc.tile_pool(name="src", bufs=1) as src_pool,
            tc.tile_pool(name="wp", bufs=1) as w_pool,
            tc.tile_pool(name="i2c", bufs=8) as i2c_pool,
            tc.tile_pool(name="ob", bufs=3) as ob_pool,
            tc.tile_pool(name="ps", bufs=8, space="PSUM") as ps_pool,
        ):
            srct = src_pool.tile([128, 2 * SLAB], mybir.dt.float16)
            wt = w_pool.tile([128, OC], mybir.dt.float16)

            p_stride = srct.ap[0][0]  # partition pitch in elements

            nc.sync.dma_start(out=wt[0:49, :], in_=wT[:, :])
            nc.sync.dma_start(out=wt[64:113, :], in_=wT[:, :])
            for slot in range(2):
                nc.sync.dma_start(
                    out=srct[:, slot * SLAB:(slot + 1) * SLAB],
                    in_=xb[slot, :, :])

            # software-pipelined emission: issue im2col DMAs PREFETCH pairs
            # ahead of the compute stream so the POOL engine's in-order
            # instruction stream never blocks descriptor emission on a
            # downstream dependency.
            PREFETCH = 7
            i2c_tiles = {}

            def issue_i2c(q):
                i2c = i2c_pool.tile([128, NCOLS], mybir.dt.float16,
                                    tag="i2c", name=f"i2c{q}")
                for half in range(2):
                    t = 2 * q + half
                    base, slot = _tile_src(t)
                    src = bass.AP(
                        tensor=srct.tensor,
                        offset=srct.offset + base * p_stride + slot * SLAB,
                        ap=[[4 * p_stride, KS], [1, KS], [1, NCOLS]],
                    )
                    b0 = 64 * half
                    nc.gpsimd.dma_start(
                        out=i2c[b0:b0 + KS * KS, :], in_=src)
                i2c_tiles[q] = i2c

            for q in range(min(PREFETCH, NPAIRS)):
                issue_i2c(q)

            for q in range(NPAIRS):
                if q + PREFETCH < NPAIRS:
                    issue_i2c(q + PREFETCH)
                i2c = i2c_tiles.pop(q)

                ob = ob_pool.tile([128, NCOLS], mybir.dt.float16, tag="ob")
                for j in range(NMM):
                    ps = ps_pool.tile([128, 448], mybir.dt.float32, tag="ps")
                    nc.tensor.matmul(
                        ps[0:OC, :], wt[0:49, :],
                        i2c[0:49, 448 * j: 448 * (j + 1)],
                        start=True, stop=True)
                    nc.tensor.matmul(
                        ps[OC:128, :], wt[64:113, :],
                        i2c[64:113, 448 * j: 448 * (j + 1)],
                        start=True, stop=True)
                    if j % 2 == 0:
                        nc.vector.tensor_copy(
                            ob[:, 448 * j: 448 * (j + 1)], ps[:, :])
                    else:
                        nc.scalar.copy(
                            ob[:, 448 * j: 448 * (j + 1)], ps[:, :])

                # fp16 -> fp32 cast during the store; only SWDGE casts
                for half in range(2):
                    t = 2 * q + half
                    img, blk = divmod(t, NBLK)
                    r0 = BLK * blk
                    nrows = min(BLK, OH - r0)
                    nc.gpsimd.dma_start(
                        out=out[img, :, r0: r0 + nrows, :],
                        in_=ob[64 * half: 64 * half + OC, : nrows * OW])

    nc.compile()
    return nc


def _prep_inputs(data, weight):
    d16 = np.ascontiguousarray(data.reshape(B, H, H)).astype(np.float16)
    dpad = np.zeros((B, 256, H), dtype=np.float16)
    dpad[:, :H, :] = d16
    wt = np.ascontiguousarray(
        weight.reshape(OC, KS * KS).T).astype(np.float16)

    in_maps = []
    for c in range(NCORES):
        xb = np.zeros((2, 128, SLAB), dtype=np.float16)
        for t in range(NTILES):
            img, blk = divmod(t, NBLK)
            gimg = c * IPC + img
            base, slot = _tile_src(t)
            for ky in range(KS):
                r0 = BLK * blk + ky
                xb[slot, base + 4 * ky, : SRC_ROWS * H] = \
                    dpad[gimg, r0: r0 + SRC_ROWS, :].ravel()
        in_maps.append({"xb": xb, "wT": wt})
    return in_maps


def kernel(data, weight):
    from concourse.bass_utils import run_bass_kernel_spmd

    if "nc" not in _CACHE:
        _CACHE["nc"] = _build()
    nc = _CACHE["nc"]

    in_maps = _prep_inputs(np.asarray(data), np.asarray(weight))
    res = run_bass_kernel_spmd(nc, in_maps, core_ids=list(range(NCORES)))
    outs = [r["out"] for r in res.results]
    full = np.concatenate(outs, axis=0)  # [32, 64, 218, 224]
    return np.ascontiguousarray(full[:, :, :, :OH]).astype(np.float32)


# revision 14
# speedup vs baseline: 1.0075x; 1.0075x over previous
"""Trainium2 Bass kernel: Conv2d(1->64, k=7, valid) on data [32,1,224,224] f32.

Data-parallel over batch (4 images per core on 8 cores).  Per core:
im2col matmul in fp16 (K=49 taps, M=64 out-channels), PSUM fp32, fp32 out.

Layout/pipeline (per core, 32 row-block "tiles" of 28 output rows):
  - host: fp16 cast; for each tile, SEVEN copies of its 34-row source
    block, copy ky pre-shifted down by ky rows.  Copies of one tile sit at
    partitions base+4*ky (7 distinct AXI ports); even tiles use the lower
    partition half / even ports, odd tiles the upper half / odd ports.
  - im2col: ONE SWDGE DMA per tile (3-dim AP): src dim0 walks the 7 slab
    copies, dim1 the 7 kx shifts (overlapping reads), dim2 a contiguous
    28*224-col run.  dst = [49, 6272] fp16 at partition base 0 (even
    tiles) or 64 (odd tiles).
  - matmul: pairs (even tile, odd tile): lhsT = W^T [49,64] fp16 at row
    base 0/64, out -> psum[0:64]/[64:128] of one bank.  Alternating row
    groups lets LDWEIGHTS overlap in-flight matmuls.
  - copy: psum [128,448] -> ob tile full width, DVE/ACT alternating.
  - out: one DMA per tile [64ch, 28*224 f32]; even tiles on the sync
    HWDGE ring, odd on scalar.  Cols 218..223 are garbage (kx wrap) and
    are sliced off on the host, as are rows >= 218.
"""

import numpy as np

B = 32            # full batch
OC = 64           # out channels
KS = 7            # kernel size
H = 224           # input H=W
OH = 218          # valid output rows/cols
OW = 224          # computed output width (incl 6 garbage cols)
NCORES = 8
IPC = B // NCORES  # images per core

BLK = 28          # output rows per tile
NBLK = 8          # tiles per image
SRC_ROWS = 34     # rows stored per slab copy
SLAB = SRC_ROWS * H + 8   # 7624 fp16 elements per slab
NTILES = IPC * NBLK       # 32 tiles per core
NPAIRS = NTILES // 2
NCOLS = BLK * OW          # 6272 im2col columns per tile
NMM = NCOLS // 448        # 14 matmuls per tile

# slab-group bases: tile t -> 7 slabs at partitions base+4*ky, where
# base = (64 if t odd) + BASES[(t//2) % 8], free slot (t//2) // 8.
BASES = [0, 1, 2, 3, 28, 29, 30, 31]

_CACHE = {}


def _tile_src(t):
    q = t // 2
    base = BASES[q % 8] + (64 if (t % 2) else 0)
    return base, q // 8  # partition base, slot


def _build():
    import concourse.bass as bass
    import concourse.mybir as mybir
    import concourse.tile as tile
    from concourse import bacc

    nc = bacc.Bacc("TRN2", target_bir_lowering=False, debug=False)

    xb = nc.dram_tensor("xb", [2, 128, SLAB], mybir.dt.float16,
                        kind="ExternalInput")
    wT = nc.dram_tensor("wT", [KS * KS, OC], mybir.dt.float16,
                        kind="ExternalInput")
    out = nc.dram_tensor("out", [IPC, OC, OH, OW], mybir.dt.float32,
                         kind="ExternalOutput")

    with tile.TileContext(nc) as tc:
        with (
            tc.tile_pool(name="src", bufs=1) as src_pool,
            tc.tile_pool(name="wp", bufs=1) as w_pool,
            tc.tile_pool(name="i2c", bufs=8) as i2c_pool,
            tc.tile_pool(name="ob", bufs=3) as ob_pool,
            tc.tile_pool(name="ps", bufs=8, space="PSUM") as ps_pool,
        ):
            srct = src_pool.tile([128, 2 * SLAB], mybir.dt.float16)
            wt = w_pool.tile([128, OC], mybir.dt.float16)

            p_stride = srct.ap[0][0]  # partition pitch in elements

            nc.sync.dma_start(out=wt[0:49, :], in_=wT[:, :])
            nc.sync.dma_start(out=wt[64:113, :], in_=wT[:, :])
            for slot in range(2):
                nc.sync.dma_start(
                    out=srct[:, slot * SLAB:(slot + 1) * SLAB],
                    in_=xb[slot, :, :])

            # software-pipelined emission: issue im2col DMAs PREFETCH pairs
            # ahead of the compute stream so the POOL engine's in-order
            # instruction stream never blocks descriptor emission on a
            # downstream dependency.
            PREFETCH = 7
            i2c_tiles = {}

            def issue_i2c(q):
                i2c = i2c_pool.tile([128, NCOLS], mybir.dt.float16,
                                    tag="i2c", name=f"i2c{q}")
                for half in range(2):
                    t = 2 * q + half
                    base, slot = _tile_src(t)
                    src = bass.AP(
                        tensor=srct.tensor,
                        offset=srct.offset + base * p_stride + slot * SLAB,
                        ap=[[4 * p_stride, KS], [1, KS], [1, NCOLS]],
                    )
                    b0 = 64 * half
                    nc.gpsimd.dma_start(
                        out=i2c[b0:b0 + KS * KS, :], in_=src)
                i2c_tiles[q] = i2c

            for q in range(min(PREFETCH, NPAIRS)):
                issue_i2c(q)

            for q in range(NPAIRS):
                if q + PREFETCH < NPAIRS:
                    issue_i2c(q + PREFETCH)
                i2c = i2c_tiles.pop(q)

                ob = ob_pool.tile([128, NCOLS], mybir.dt.float16, tag="ob")
                for j in range(NMM):
                    ps = ps_pool.tile([128, 448], mybir.dt.float32, tag="ps")
                    nc.tensor.matmul(
                        ps[0:OC, :], wt[0:49, :],
                        i2c[0:49, 448 * j: 448 * (j + 1)],
                        start=True, stop=True)
                    nc.tensor.matmul(
                        ps[OC:128, :], wt[64:113, :],
                        i2c[64:113, 448 * j: 448 * (j + 1)],
                        start=True, stop=True)
                    if j % 2 == 0:
                        nc.vector.tensor_copy(
                            ob[:, 448 * j: 448 * (j + 1)], ps[:, :])
                    else:
                        nc.scalar.copy(
                            ob[:, 448 * j: 448 * (j + 1)], ps[:, :])

                # fp16 -> fp32 cast during the store; only SWDGE casts
                for half in range(2):
                    t = 2 * q + half
                    img, blk = divmod(t, NBLK)
                    r0 = BLK * blk
                    nrows = min(BLK, OH - r0)
                    nc.gpsimd.dma_start(
                        out=out[img, :, r0: r0 + nrows, :],
                        in_=ob[64 * half: 64 * half + OC, : nrows * OW])

    nc.compile()
    return nc


def _prep_inputs(data, weight):
    d16 = np.ascontiguousarray(data.reshape(B, H, H)).astype(np.float16)
    dpad = np.zeros((B, 256, H), dtype=np.float16)
    dpad[:, :H, :] = d16
    wt = np.ascontiguousarray(
        weight.reshape(OC, KS * KS).T).astype(np.float16)

    in_maps = []
    for c in range(NCORES):
        xb = np.zeros((2, 128, SLAB), dtype=np.float16)
        for t in range(NTILES):
            img, blk = divmod(t, NBLK)
            gimg = c * IPC + img
            base, slot = _tile_src(t)
            for ky in range(KS):
                r0 = BLK * blk + ky
                xb[slot, base + 4 * ky, : SRC_ROWS * H] = \
                    dpad[gimg, r0: r0 + SRC_ROWS, :].ravel()
        in_maps.append({"xb": xb, "wT": wt})
    return in_maps


def kernel(data, weight):
    from concourse.bass_utils import run_bass_kernel_spmd

    if "nc" not in _CACHE:
        _CACHE["nc"] = _build()
    nc = _CACHE["nc"]

    in_maps = _prep_inputs(np.asarray(data), np.asarray(weight))
    res = run_bass_kernel_spmd(nc, in_maps, core_ids=list(range(NCORES)))
    outs = [r["out"] for r in res.results]
    full = np.concatenate(outs, axis=0)  # [32, 64, 218, 224]
    return np.ascontiguousarray(full[:, :, :, :OH]).astype(np.float32)
